# revision 1
# baseline (speedup 1.0000x reference)
"""Distributed Trainium2 (Bass) kernel for nn_Attention_53764400611491.

The reference module has HEADS == C == 64, so head_dim d = C//HEADS = 1.
With d = 1 the attention algebra collapses: per (batch b, head c)

    attn = q k^T            (outer product, [N,N])
    o    = attn @ v  =  q * (k . v)        <- a scalar per (b,c)!

so the whole module is

    out[b,c,n] = sum_c' wp[c,c'] * q[b,c',n] * s[b,c'] + x[b,c,n]
    q = wq @ x_b          s[b,c'] = sum_n (wk@x_b)[c',n] * (wv@x_b)[c',n]

and the [b,h,N,N] attention tensor never needs to exist.  With
u = (wk+wv)/2 @ x and d = (wk-wv)/2 @ x:   s = sum u^2 - sum d^2,
which lets the scalar (ACT) engine square straight out of PSUM (engines
may read at most one non-scalar PSUM operand per instruction).

Sharding over 8 NeuronCores: core i handles batch b = i//4 and output
n-chunk j = i%4 (256 of the 1024 flattened h*w positions).  Each core
receives the full x_b (rotated so its own chunk comes first), computes
s_b redundantly, and writes its 64x256 output chunk.  No collectives.

Pipeline: x and the packed weights move over the two HWDGE DMA rings
(SP + ACT) in 6 transfers, all hoisted into the BIR 'main' preamble
block (post-build instruction surgery, see _hoist_input_dmas) so the
~2us DMA completion latency overlaps the fixed NEFF preamble.  uv
matmuls, ACT squares and DVE row-sum reduces pipeline chunk by chunk;
s4 = rowsum_u - rowsum_d via a cross-partition tensor_scalar; one
tensor_scalar scales wp^T by s; the final matmul accumulates onto an
identity-preloaded "+x" PSUM bank; the Block-exit barrier semaphores
are trimmed (the walrus postamble has its own rendezvous).
"""
import numpy as np

import concourse.bass as bass
import concourse.mybir as mybir
from concourse.bass_utils import run_bass_kernel_spmd

F32 = mybir.dt.float32
F32R = mybir.dt.float32r
MULT = mybir.AluOpType.mult
SUB = mybir.AluOpType.subtract
ADD = mybir.AluOpType.add
SQUARE = mybir.ActivationFunctionType.Square
COPY = mybir.ActivationFunctionType.Copy

B, C, H, W = 2, 64, 32, 32
N = H * W          # 1024
NCHUNK = N // 4    # 256 output columns per core


HOIST_INPUT_DMAS = True
TRIM_END_BARRIER = True  # drop the Block-exit barrier semaphores (keep drains)


def _hoist_input_dmas(nc: bass.Bass, insts) -> None:
    """Move the input-DMA issue instructions from the Block body into the
    'main' preamble block, right after each engine's register-init moves.

    The BIR main block executes ~1.3us before the Block bodies (const
    memsets + all-engine barrier + branch sit in between), so hoisted
    DMA issues overlap that framework time and their completions land
    earlier.  Per-engine program order is preserved: hoisted DMAs have
    no waits, and their completion semaphores start at zero (no
    sem-clear instructions exist in this lowering mode).
    """
    main = nc.main_func.blocks[0]
    to_move = {id(i) for i in insts}
    for b in nc.main_func.blocks[1:]:
        b.instructions[:] = [i for i in b.instructions if id(i) not in to_move]
    # insertion anchor per engine: after the last InstRegisterMove
    anchor = {}
    for k, mi in enumerate(main.instructions):
        if isinstance(mi, mybir.InstRegisterMove):
            anchor[mi.engine] = k
    for inst in insts:  # keep per-engine relative order
        idx = anchor.get(inst.engine)
        assert idx is not None, f"no register-move anchor for {inst.engine}"
        main.instructions.insert(idx + 1, inst)
        for eng in anchor:
            if anchor[eng] >= idx + 1:
                anchor[eng] += 1
        anchor[inst.engine] = idx + 1


def _hoist_pe_warm(nc: bass.Bass, insts) -> None:
    """Move the PE warm-up matmuls into 'main', right AFTER the PE drain
    (which carries the barrier gather-increment) and BEFORE the PE release
    wait.  They execute during the all-engine barrier stall: the gather is
    not delayed, the release wait is long satisfied when PE reaches it, and
    ~3.3us of sustained PE activity flips the HAM clock gate to 2.4 GHz
    before the real matmuls issue."""
    main = nc.main_func.blocks[0]
    to_move = {id(i) for i in insts}
    for b in nc.main_func.blocks[1:]:
        b.instructions[:] = [i for i in b.instructions if id(i) not in to_move]
    idx = None
    for k, mi in enumerate(main.instructions):
        if isinstance(mi, mybir.InstDrain) and mi.engine == mybir.EngineType.PE:
            idx = k
            break
    assert idx is not None, "no PE drain anchor in main"
    for j, inst in enumerate(insts):
        main.instructions.insert(idx + 1 + j, inst)


def _build_nc() -> bass.Bass:
    nc = bass.Bass()
    x_ext = nc.declare_dram_parameter("xr", [128, 512], F32R, isOutput=False)
    w_ext = nc.declare_dram_parameter("w", [128, 256], F32R, isOutput=False)
    # out chunk [64,256] packed as [128,128]: partitions 0-63 = cols 0-127,
    # partitions 64-127 = cols 128-255 (full-width single DMA)
    o_ext = nc.declare_dram_parameter("out", [128, 128], F32, isOutput=True)

    from contextlib import ExitStack

    with ExitStack() as ctx:
        e = ctx.enter_context
        # Wsb cols: 0:128 wkv (u/d weights, duplicated over partition halves)
        #           128:192 wq.T (parts 0:64) / wp.T (parts 64:128)
        #           192:256 eye (parts 0:64) / 0
        Wsb = e(nc.sbuf_tensor("Wsb", [128, 256], F32R))
        Xsb = e(nc.sbuf_tensor("Xsb", [128, 512], F32R))
        sqs = e(nc.sbuf_tensor("sqs", [128, 1088], F32))  # squared u/d chunks
        redc = e(nc.sbuf_tensor("redc", [128, 4], F32))   # per-chunk row sums
        redall = e(nc.sbuf_tensor("redall", [128, 1], F32))
        s4 = e(nc.sbuf_tensor("s4", [64, 1], F32))
        Qsb = e(nc.sbuf_tensor("Qsb", [64, 256], F32R))
        wpTs = e(nc.sbuf_tensor("wpTs", [64, 64], F32R))
        Fsb = e(nc.sbuf_tensor("Fsb", [64, 128], F32))
        Ftmp = e(nc.sbuf_tensor("Ftmp", [64, 128], F32))
        dummy = e(nc.sbuf_tensor("warmup", [1, 1], F32))
        uv1 = e(nc.psum_tensor("uv1", [128, 320], F32))
        uv2 = e(nc.psum_tensor("uv2", [128, 192], F32))
        uv3 = e(nc.psum_tensor("uv3", [128, 384], F32))
        uv4 = e(nc.psum_tensor("uv4", [128, 128], F32))
        Qp = e(nc.psum_tensor("Qp", [64, 256], F32))
        Op = e(nc.psum_tensor("Op", [64, 256], F32))
        w_sem = e(nc.semaphore("w_sem"))
        xa1_sem = e(nc.semaphore("xa1_sem"))
        xa2_sem = e(nc.semaphore("xa2_sem"))
        xb1_sem = e(nc.semaphore("xb1_sem"))
        xb2_sem = e(nc.semaphore("xb2_sem"))
        pe_sem = e(nc.semaphore("pe_sem"))
        dv_sem = e(nc.semaphore("dv_sem"))
        act_sem = e(nc.semaphore("act_sem"))
        out_sem = e(nc.semaphore("out_sem"))
        block = e(nc.Block())

        def r(ap):
            return ap.bitcast(F32R)

        hoist = []

        @block.sync
        def _(sync):
            # two hoisted gens on the SP HWDGE ring: its pre-barrier drain
            # does not wait for transfer completion (the Pool/SWDGE one does),
            # and 2 gens (~1.3us) fit inside the preamble slack before the
            # all-engine barrier would otherwise release.
            hoist.append(sync.dma_start(Xsb[0:64, 0:320], x_ext[0:64, 0:320]).then_inc(xa1_sem, 16))
            hoist.append(sync.dma_start(Xsb[64:128, 0:384], x_ext[64:128, 0:384]).then_inc(xb1_sem, 16))
            hoist.append(sync.dma_start(Xsb[0:64, 320:512], x_ext[0:64, 320:512]).then_inc(xa2_sem, 16))
            sync.wait_ge(dv_sem, 3)
            sync.dma_start(o_ext[0:64, :], Fsb[:]).then_inc(out_sem, 16)
            sync.wait_ge(out_sem, 32)

        @block.tensor
        def _(pe):
            pe.wait_ge(w_sem, 16)
            pe.wait_ge(xa1_sem, 16)
            # u,d chunks: rows 0-63 = u = (wk+wv)x/2, rows 64-127 = d = (wk-wv)x/2
            # All four uv matmuls run back-to-back: they feed the ACT square
            # chain, which paces the tail.  q is not needed until the final
            # matmul, so Qp runs after them, off the critical path.
            pe.matmul(uv1[:], r(Wsb[0:64, 0:128]), r(Xsb[0:64, 0:320]), start=True, stop=True).then_inc(pe_sem, 1)
            pe.wait_ge(xb1_sem, 16)
            pe.matmul(uv3[:], r(Wsb[64:128, 0:128]), r(Xsb[64:128, 0:384]), start=True, stop=True).then_inc(pe_sem, 1)
            pe.wait_ge(xa2_sem, 16)
            pe.matmul(uv2[:], r(Wsb[0:64, 0:128]), r(Xsb[0:64, 320:512]), start=True, stop=True).then_inc(pe_sem, 1)
            pe.wait_ge(xb2_sem, 16)
            pe.matmul(uv4[:], r(Wsb[64:128, 0:128]), r(Xsb[64:128, 384:512]), start=True, stop=True).then_inc(pe_sem, 1)
            # q for own chunk (xa1 covers cols 0:320 > 0:256)
            pe.wait_ge(w_sem, 32)
            pe.matmul(Qp[:], r(Wsb[0:64, 128:192]), r(Xsb[0:64, 0:256]), start=True, stop=True).then_inc(pe_sem, 1)
            # preload x chunk into the output PSUM bank (identity matmul)
            pe.matmul(Op[:], r(Wsb[0:64, 192:256]), r(Xsb[0:64, 0:256]), start=True, stop=False).then_inc(pe_sem, 1)
            # out = (wp diag(s)) @ q + x  (accumulates into Op)
            pe.wait_ge(dv_sem, 2)
            pe.wait_ge(act_sem, 6)
            pe.matmul(Op[:], r(wpTs[:]), r(Qsb[:]), start=False, stop=True).then_inc(pe_sem, 1)

        @block.scalar
        def _(act):
            # wkv half first: the first uv matmul only needs it
            hoist.append(act.dma_start(Wsb[:, 0:128], w_ext[:, 0:128]).then_inc(w_sem, 16))
            hoist.append(act.dma_start(Wsb[:, 128:256], w_ext[:, 128:256]).then_inc(w_sem, 16))
            hoist.append(act.dma_start(Xsb[64:128, 384:512], x_ext[64:128, 384:512]).then_inc(xb2_sem, 16))
            # explicit activation-table load (set 0 holds Square); placing it
            # ourselves keeps the compiler from inserting one later and skips
            # the warm-up activation entirely
            act.add_instruction(mybir.InstLoadActFuncSet(
                name=nc.get_next_instruction_name(), act_func_set_id=0, ins=[], outs=[],
            )).then_inc(act_sem, 1)
            act.wait_ge(pe_sem, 1)
            act.activation(sqs[:, 0:320], uv1[:], SQUARE).then_inc(act_sem, 1)
            act.wait_ge(pe_sem, 2)
            act.activation(sqs[:, 384:768], uv3[:], SQUARE).then_inc(act_sem, 1)
            act.wait_ge(pe_sem, 3)
            act.activation(sqs[:, 768:960], uv2[:], SQUARE).then_inc(act_sem, 1)
            act.wait_ge(pe_sem, 4)
            act.activation(sqs[:, 960:1088], uv4[:], SQUARE).then_inc(act_sem, 1)
            # q copy PSUM->SBUF (with f32r rounding); ACT is idle here and
            # DVE is busy with the reduction tail
            act.wait_ge(pe_sem, 5)
            act.activation(Qsb[:], Qp[:], COPY).then_inc(act_sem, 1)
            act.wait_ge(dv_sem, 4)  # DVE wrote Ftmp
            act.dma_start(o_ext[64:128, :], Ftmp[:]).then_inc(out_sem, 16)

        @block.vector
        def _(dv):
            dv.wait_ge(w_sem, 32)  # wpTs reads the W2 half
            # per-chunk row sums, each behind its square
            dv.wait_ge(act_sem, 2)
            dv.reduce_sum(redc[:, 0:1], sqs[:, 0:320], axis=mybir.AxisListType.X)
            dv.wait_ge(act_sem, 3)
            dv.reduce_sum(redc[:, 2:3], sqs[:, 384:768], axis=mybir.AxisListType.X)
            dv.wait_ge(act_sem, 4)
            dv.reduce_sum(redc[:, 1:2], sqs[:, 768:960], axis=mybir.AxisListType.X)
            dv.wait_ge(act_sem, 5)
            dv.reduce_sum(redc[:, 3:4], sqs[:, 960:1088], axis=mybir.AxisListType.X)
            dv.drain()  # redc landed (same-engine RAW)
            dv.reduce_sum(redall[:], redc[:], axis=mybir.AxisListType.X)
            dv.drain()  # redall landed
            # s4 = sum u^2 - sum d^2  (cross-base scalar operand)
            dv.tensor_scalar(s4[:], redall[0:64, :], redall[64:128, :], None, op0=SUB).then_inc(dv_sem, 1)
            dv.drain()  # s4 landed
            # wpTs = wp.T * s
            dv.tensor_scalar(wpTs[:], Wsb[64:128, 128:192], s4[:], None, op0=MULT).then_inc(dv_sem, 1)
            # out chunk halves PSUM -> SBUF
            dv.wait_ge(pe_sem, 7)
            dv.tensor_copy(Fsb[:], Op[:, 0:128]).then_inc(dv_sem, 1)
            dv.tensor_copy(Ftmp[:], Op[:, 128:256]).then_inc(dv_sem, 1)

    if HOIST_INPUT_DMAS:
        _hoist_input_dmas(nc, [h.ins for h in hoist])
    if TRIM_END_BARRIER:
        # the walrus postamble has its own all-engine rendezvous; the Block
        # exit barrier only delays it.  Keep the drains (write fences).
        end = nc.main_func.blocks[-1]
        end.instructions[:] = [
            i for i in end.instructions if not isinstance(i, mybir.InstEventSemaphore)
        ]
    return nc


def _shard_inputs(x, wq, wk, wv, wp):
    """Full inputs -> list of 8 per-core {'xr','w'} dicts."""
    x = np.asarray(x, dtype=np.float32)
    wq, wk, wv, wp = (np.asarray(a, dtype=np.float32) for a in (wq, wk, wv, wp))
    xf = np.ascontiguousarray(x.reshape(B, C, N))
    kv = np.concatenate([(wk + wv).T, (wk - wv).T], axis=1) * 0.5    # [64,128]
    wkv = np.concatenate([kv, kv], axis=0)                           # [128,128]
    eye = np.eye(64, dtype=np.float32)
    zero = np.zeros((64, 64), dtype=np.float32)
    wqp = np.concatenate(
        [np.concatenate([wq.T, eye], axis=1),
         np.concatenate([wp.T, zero], axis=1)], axis=0)              # [128,128]
    wfull = np.ascontiguousarray(np.concatenate([wkv, wqp], axis=1))  # [128,256]
    in_maps = []
    for core in range(8):
        bb, j = core // 4, core % 4
        chunks = [xf[bb, :, ((j + t) % 4) * NCHUNK:(((j + t) % 4) + 1) * NCHUNK] for t in range(4)]
        upper = np.concatenate(chunks[0:2], axis=1)  # [64,512]
        lower = np.concatenate(chunks[2:4], axis=1)  # [64,512]
        xr = np.ascontiguousarray(np.concatenate([upper, lower], axis=0))  # [128,512]
        in_maps.append({"xr": xr, "w": wfull})
    return in_maps


def _gather_outputs(results):
    """8 per-core {'out': [128,128]} -> full [b,C,h,w].

    Per-core out is the [64,256] chunk packed as [128,128]:
    partitions 0-63 = cols 0-127, partitions 64-127 = cols 128-255.
    """
    out = np.empty((B, C, N), dtype=np.float32)
    for core in range(8):
        bb, j = core // 4, core % 4
        o = np.asarray(results[core]["out"])
        chunk = np.concatenate([o[0:64, :], o[64:128, :]], axis=1)  # [64,256]
        out[bb, :, j * 256:(j + 1) * 256] = chunk
    return out.reshape(B, C, H, W)


_NC_CACHE = None


def kernel(x, wq, wk, wv, wp) -> np.ndarray:
    global _NC_CACHE
    if _NC_CACHE is None:
        _NC_CACHE = _build_nc()
    in_maps = _shard_inputs(x, wq, wk, wv, wp)
    last_err = None
    for _ in range(3):
        try:
            res = run_bass_kernel_spmd(_NC_CACHE, in_maps, core_ids=list(range(8)))
            return _gather_outputs(res.results)
        except Exception as exc:  # transient device-unrecoverable resets on retry
            last_err = exc
    raise last_err



# revision 11
# speedup vs baseline: 1.0663x; 1.0663x over previous
"""Distributed Trainium2 (Bass) kernel for nn_Attention_53764400611491.

The reference module has HEADS == C == 64, so head_dim d = C//HEADS = 1.
With d = 1 the attention algebra collapses: per (batch b, head c)

    attn = q k^T            (outer product, [N,N])
    o    = attn @ v  =  q * (k . v)        <- a scalar per (b,c)!

so the whole module is

    out[b,c,n] = sum_c' wp[c,c'] * q[b,c',n] * s[b,c'] + x[b,c,n]
    q = wq @ x_b          s[b,c'] = sum_n (wk@x_b)[c',n] * (wv@x_b)[c',n]

and the [b,h,N,N] attention tensor never needs to exist.  With
u = (wk+wv)/2 @ x and d = (wk-wv)/2 @ x:   s = sum u^2 - sum d^2,
which keeps every reduction input to a single PSUM operand (hardware
allows at most one non-scalar PSUM input per instruction; two matmuls
may NOT share a PSUM bank - that wedges the device).

Sharding over 8 NeuronCores: core i handles batch b = i//4 and output
n-chunk j = i%4 (256 of the 1024 flattened h*w positions).  Each core
receives the full x_b (rotated so its own chunk comes first), computes
s_b redundantly, and writes its 64x256 output chunk.  No collectives.

v3 schedule (vs the 18.4us baseline):
 - Input DMAs are the FIRST bass instructions in 'main' (before the
   register-init moves): x halves on the SP HWDGE ring, w on the ACT
   ring, so they complete ~8.3us - right as the Block bodies open
   (the fixed walrus/NRT preamble runs until ~5.7us regardless).
 - All f32r matmuls use >=256 moving columns (1 cycle/row; below 256
   the PE runs 2-4x slower per row).  One matmul per [128,256] PSUM
   bank: uv1/uv3 = partition-half A of x1/x2, uv2/uv4 = half B.
 - Square + row-sum is fused into single instructions: ACT
   activation(Square, accum_out=) on chunks 1,3,4; DVE covers chunk 2
   (tensor_copy to SBUF + scalar_tensor_tensor mult/mult accum) so the
   serial ACT chain is 3 chunks, not 4.
 - The first three partials pre-combine on DVE while ACT squares the
   last chunk; the final combine is one two-scalar tensor_scalar.
 - wp^T scaled by s; one final matmul; DVE adds +x while copying
   PSUM->SBUF; single [64,256] output DMA on the SP ring.
 - The out_sem wait lives on the TENSOR engine: its slot is last in the
   walrus exit rendezvous, so the other engines' exit work overlaps the
   output-DMA completion latency.
"""
import numpy as np

import concourse.bass as bass
import concourse.mybir as mybir
from concourse.bass_utils import run_bass_kernel_spmd

F32 = mybir.dt.float32
F32R = mybir.dt.float32r
MULT = mybir.AluOpType.mult
SUB = mybir.AluOpType.subtract
ADD = mybir.AluOpType.add
SQUARE = mybir.ActivationFunctionType.Square
COPY = mybir.ActivationFunctionType.Copy

B, C, H, W = 2, 64, 32, 32
N = H * W          # 1024
NCHUNK = N // 4    # 256 output columns per core


TRIM_END_BARRIER = True  # drop the Block-exit barrier semaphores (keep drains)
HOIST_FRONT = True       # front of main (vs baseline's after-register-moves anchor)


def _hoist_to_front(nc: bass.Bass, insts) -> None:
    """Move the given instructions to the very front of the 'main' preamble
    block (right after the leading dummy call), preserving their relative
    order.  They execute as each engine's first instructions, so input-DMA
    completions land before the Block bodies open.  Hoisted instructions
    must have no semaphore waits."""
    main = nc.main_func.blocks[0]
    to_move = {id(i) for i in insts}
    for b in nc.main_func.blocks[1:]:
        b.instructions[:] = [i for i in b.instructions if id(i) not in to_move]
    main.instructions[:] = [i for i in main.instructions if id(i) not in to_move]
    pos = 1 if main.instructions and isinstance(main.instructions[0], mybir.InstCall) else 0
    for j, inst in enumerate(insts):
        main.instructions.insert(pos + j, inst)


def _hoist_after_moves(nc: bass.Bass, insts) -> None:
    """Baseline-style hoist: insert after each engine's last register-init
    move in 'main', preserving per-engine relative order."""
    main = nc.main_func.blocks[0]
    to_move = {id(i) for i in insts}
    for b in nc.main_func.blocks[1:]:
        b.instructions[:] = [i for i in b.instructions if id(i) not in to_move]
    main.instructions[:] = [i for i in main.instructions if id(i) not in to_move]
    anchor = {}
    for k, mi in enumerate(main.instructions):
        if isinstance(mi, mybir.InstRegisterMove):
            anchor[mi.engine] = k
    for inst in insts:
        idx = anchor.get(inst.engine)
        assert idx is not None, f"no register-move anchor for {inst.engine}"
        main.instructions.insert(idx + 1, inst)
        for eng in anchor:
            if anchor[eng] >= idx + 1:
                anchor[eng] += 1
        anchor[inst.engine] = idx + 1


def _build_nc() -> bass.Bass:
    nc = bass.Bass()
    x_ext = nc.declare_dram_parameter("xr", [128, 512], F32R, isOutput=False)
    w_ext = nc.declare_dram_parameter("w", [128, 192], F32R, isOutput=False)
    o_ext = nc.declare_dram_parameter("out", [64, 256], F32, isOutput=True)

    from contextlib import ExitStack

    with ExitStack() as ctx:
        e = ctx.enter_context
        # Wsb cols: 0:128 wkv (u/d weights, duplicated over partition halves)
        #           128:192 wq.T (parts 0:64) / wp.T (parts 64:128)
        Wsb = e(nc.sbuf_tensor("Wsb", [128, 192], F32R))
        Xsb = e(nc.sbuf_tensor("Xsb", [128, 512], F32R))
        cp2 = e(nc.sbuf_tensor("cp2", [128, 256], F32))   # SBUF copy of uv2
        sqs = e(nc.sbuf_tensor("sqs", [128, 256], F32))   # DVE square out (unread)
        redc = e(nc.sbuf_tensor("redc", [128, 4], F32))   # per-chunk row sums
        redall3 = e(nc.sbuf_tensor("redall3", [128, 1], F32))
        sdiff3 = e(nc.sbuf_tensor("sdiff3", [64, 1], F32))
        s4 = e(nc.sbuf_tensor("s4", [64, 1], F32))
        Qsb = e(nc.sbuf_tensor("Qsb", [64, 256], F32R))
        wpTs = e(nc.sbuf_tensor("wpTs", [64, 64], F32R))
        Fsb = e(nc.sbuf_tensor("Fsb", [64, 256], F32))
        uv1 = e(nc.psum_tensor("uv1", [128, 256], F32))
        uv2 = e(nc.psum_tensor("uv2", [128, 256], F32))
        uv3 = e(nc.psum_tensor("uv3", [128, 256], F32))
        uv4 = e(nc.psum_tensor("uv4", [128, 256], F32))
        sqp = e(nc.psum_tensor("sqp", [128, 256], F32))   # ACT square outs (unread)
        Qp = e(nc.psum_tensor("Qp", [64, 256], F32))
        Of = e(nc.psum_tensor("Of", [64, 256], F32))
        x1_sem = e(nc.semaphore("x1_sem"))
        x2_sem = e(nc.semaphore("x2_sem"))
        w_sem = e(nc.semaphore("w_sem"))
        pe_sem = e(nc.semaphore("pe_sem"))
        act_sem = e(nc.semaphore("act_sem"))
        dv_sem = e(nc.semaphore("dv_sem"))
        out_sem = e(nc.semaphore("out_sem"))
        block = e(nc.Block())

        def r(ap):
            return ap.bitcast(F32R)

        hoist = []

        @block.sync
        def _(sync):
            # input x halves on the SP HWDGE ring, hoisted to main-front
            hoist.append(sync.dma_start(Xsb[:, 0:256], x_ext[:, 0:256]).then_inc(x1_sem, 16))
            hoist.append(sync.dma_start(Xsb[:, 256:512], x_ext[:, 256:512]).then_inc(x2_sem, 16))
            # output chunk
            sync.wait_ge(dv_sem, 3)
            sync.dma_start(o_ext[:], Fsb[:]).then_inc(out_sem, 16)

        @block.tensor
        def _(pe):
            pe.wait_ge(w_sem, 16)
            pe.wait_ge(x1_sem, 16)
            # u,d: rows 0-63 = u = (wk+wv)x/2, rows 64-127 = d = (wk-wv)x/2
            pe.matmul(uv1[:], r(Wsb[0:64, 0:128]), r(Xsb[0:64, 0:256]), start=True, stop=True).then_inc(pe_sem, 1)
            pe.matmul(uv2[:], r(Wsb[64:128, 0:128]), r(Xsb[64:128, 0:256]), start=True, stop=True).then_inc(pe_sem, 1)
            # q for own chunk (cols 0:256 = own chunk, rotation puts it first)
            pe.matmul(Qp[:], r(Wsb[0:64, 128:192]), r(Xsb[0:64, 0:256]), start=True, stop=True).then_inc(pe_sem, 1)
            pe.wait_ge(x2_sem, 16)
            pe.matmul(uv3[:], r(Wsb[0:64, 0:128]), r(Xsb[0:64, 256:512]), start=True, stop=True).then_inc(pe_sem, 1)
            pe.matmul(uv4[:], r(Wsb[64:128, 0:128]), r(Xsb[64:128, 256:512]), start=True, stop=True).then_inc(pe_sem, 1)
            # out_attn = (wp diag(s)) @ q
            pe.wait_ge(dv_sem, 2)
            pe.matmul(Of[:], r(wpTs[:]), r(Qsb[:]), start=True, stop=True).then_inc(pe_sem, 1)
            # PE joins the walrus exit rendezvous LAST (slot 8): parking the
            # output-DMA completion wait here lets every other engine's exit
            # work overlap the ~1.4us DMA latency.
            pe.wait_ge(out_sem, 16)

        @block.scalar
        def _(act):
            # whole w in one ACT-ring DMA; act table load behind it (both hoisted)
            hoist.append(act.dma_start(Wsb[:], w_ext[:]).then_inc(w_sem, 16))
            hoist.append(act.add_instruction(mybir.InstLoadActFuncSet(
                name=nc.get_next_instruction_name(), act_func_set_id=0, ins=[], outs=[],
            )))
            # fused square + row-sum for chunks 1, 3, 4
            act.wait_ge(pe_sem, 1)
            act.activation(sqp[:], uv1[:], SQUARE, accum_out=redc[:, 0:1]).then_inc(act_sem, 1)
            act.wait_ge(pe_sem, 4)
            act.activation(sqp[:], uv3[:], SQUARE, accum_out=redc[:, 2:3]).then_inc(act_sem, 1)
            act.wait_ge(pe_sem, 5)
            act.activation(sqp[:], uv4[:], SQUARE, accum_out=redc[:, 3:4]).then_inc(act_sem, 1)

        @block.vector
        def _(dv):
            dv.wait_ge(w_sem, 16)  # wpTs reads Wsb
            # chunk 2 square on DVE: PSUM->SBUF copy, then square+row-sum in
            # one scalar_tensor_tensor (out = (cp*1)*cp, accum = row sum)
            dv.wait_ge(pe_sem, 2)
            dv.tensor_copy(cp2[:], uv2[:])
            dv.scalar_tensor_tensor(sqs[:], cp2[:], 1.0, cp2[:], MULT, MULT, accum_out=redc[:, 1:2])
            # q copy PSUM->SBUF (with f32r rounding)
            dv.wait_ge(pe_sem, 3)
            dv.tensor_copy(Qsb[:], Qp[:]).then_inc(dv_sem, 1)
            dv.drain()  # own redc col 1 landed
            # pre-combine partials 1,2,3 while ACT squares chunk 4
            dv.wait_ge(act_sem, 2)
            dv.reduce_sum(redall3[:], redc[:, 0:3], axis=mybir.AxisListType.X)
            dv.drain()  # redall3 landed
            dv.tensor_scalar(sdiff3[:], redall3[0:64, :], redall3[64:128, :], None, op0=SUB)
            dv.drain()  # sdiff3 landed
            # s = (partial4_u - partial4_d) + sdiff3
            dv.wait_ge(act_sem, 3)
            dv.tensor_scalar(s4[:], redc[0:64, 3:4], redc[64:128, 3:4], sdiff3[:], op0=SUB, op1=ADD)
            dv.drain()  # s4 landed
            # wpTs = wp.T * s
            dv.tensor_scalar(wpTs[:], Wsb[64:128, 128:192], s4[:], None, op0=MULT).then_inc(dv_sem, 1)
            # out = out_attn + x for own chunk, PSUM -> SBUF
            dv.wait_ge(pe_sem, 6)
            dv.tensor_tensor(Fsb[:], Of[:], Xsb[0:64, 0:256], ADD).then_inc(dv_sem, 1)

    if HOIST_FRONT:
        _hoist_to_front(nc, [h.ins for h in hoist])
    else:
        _hoist_after_moves(nc, [h.ins for h in hoist])
    if TRIM_END_BARRIER:
        # the walrus postamble has its own all-engine rendezvous; the Block
        # exit barrier only delays it.  Keep the drains (write fences).
        end = nc.main_func.blocks[-1]
        end.instructions[:] = [
            i for i in end.instructions if not isinstance(i, mybir.InstEventSemaphore)
        ]
    return nc


def _shard_inputs(x, wq, wk, wv, wp):
    """Full inputs -> list of 8 per-core {'xr','w'} dicts."""
    x = np.asarray(x, dtype=np.float32)
    wq, wk, wv, wp = (np.asarray(a, dtype=np.float32) for a in (wq, wk, wv, wp))
    xf = np.ascontiguousarray(x.reshape(B, C, N))
    kv = np.concatenate([(wk + wv).T, (wk - wv).T], axis=1) * 0.5    # [64,128]
    wkv = np.concatenate([kv, kv], axis=0)                           # [128,128]
    wqp = np.concatenate([wq.T, wp.T], axis=0)                       # [128,64]
    wfull = np.ascontiguousarray(np.concatenate([wkv, wqp], axis=1))  # [128,192]
    in_maps = []
    for core in range(8):
        bb, j = core // 4, core % 4
        chunks = [xf[bb, :, ((j + t) % 4) * NCHUNK:(((j + t) % 4) + 1) * NCHUNK] for t in range(4)]
        upper = np.concatenate(chunks[0:2], axis=1)  # [64,512]
        lower = np.concatenate(chunks[2:4], axis=1)  # [64,512]
        xr = np.ascontiguousarray(np.concatenate([upper, lower], axis=0))  # [128,512]
        in_maps.append({"xr": xr, "w": wfull})
    return in_maps


def _gather_outputs(results):
    """8 per-core {'out': [64,256]} -> full [b,C,h,w]."""
    out = np.empty((B, C, N), dtype=np.float32)
    for core in range(8):
        bb, j = core // 4, core % 4
        out[bb, :, j * 256:(j + 1) * 256] = np.asarray(results[core]["out"])
    return out.reshape(B, C, H, W)


_NC_CACHE = None


def kernel(x, wq, wk, wv, wp) -> np.ndarray:
    global _NC_CACHE
    if _NC_CACHE is None:
        _NC_CACHE = _build_nc()
    in_maps = _shard_inputs(x, wq, wk, wv, wp)
    last_err = None
    for _ in range(3):
        try:
            res = run_bass_kernel_spmd(_NC_CACHE, in_maps, core_ids=list(range(8)))
            return _gather_outputs(res.results)
        except Exception as exc:  # transient device-unrecoverable resets on retry
            last_err = exc
    raise last_err


# revision 15
# speedup vs baseline: 1.0733x; 1.0066x over previous
"""Distributed Trainium2 (Bass) kernel for nn_Attention_53764400611491.

The reference module has HEADS == C == 64, so head_dim d = C//HEADS = 1.
With d = 1 the attention algebra collapses: per (batch b, head c)

    attn = q k^T            (outer product, [N,N])
    o    = attn @ v  =  q * (k . v)        <- a scalar per (b,c)!

so the whole module is

    out[b,c,n] = sum_c' wp[c,c'] * q[b,c',n] * s[b,c'] + x[b,c,n]
    q = wq @ x_b          s[b,c'] = sum_n (wk@x_b)[c',n] * (wv@x_b)[c',n]

and the [b,h,N,N] attention tensor never needs to exist.  With
u = (wk+wv)/2 @ x and d = (wk-wv)/2 @ x:   s = sum u^2 - sum d^2,
which keeps every reduction input to a single PSUM operand (hardware
allows at most one non-scalar PSUM input per instruction; two matmuls
may NOT share a PSUM bank - that wedges the device).

Sharding over 8 NeuronCores: core i handles batch b = i//4 and output
n-chunk j = i%4 (256 of the 1024 flattened h*w positions).  Each core
receives the full x_b (rotated so its own chunk comes first), computes
s_b redundantly, and writes its 64x256 output chunk.  No collectives.

v3 schedule (vs the 18.4us baseline):
 - Input DMAs are the FIRST bass instructions in 'main' (before the
   register-init moves): x halves on the SP HWDGE ring, w on the ACT
   ring, so they complete ~8.3us - right as the Block bodies open
   (the fixed walrus/NRT preamble runs until ~5.7us regardless).
 - All f32r matmuls use >=256 moving columns (1 cycle/row; below 256
   the PE runs 2-4x slower per row).  One matmul per [128,256] PSUM
   bank: uv1/uv3 = partition-half A of x1/x2, uv2/uv4 = half B.
 - Square + row-sum is fused into single instructions: ACT
   activation(Square, accum_out=) on chunks 1,3,4; DVE covers chunk 2
   (tensor_copy to SBUF + scalar_tensor_tensor mult/mult accum) so the
   serial ACT chain is 3 chunks, not 4.
 - The first three partials pre-combine on DVE while ACT squares the
   last chunk; the final combine is one two-scalar tensor_scalar.
 - wp^T scaled by s; one final matmul; DVE adds +x while copying
   PSUM->SBUF; single [64,256] output DMA on the SP ring.
 - The out_sem wait lives on the TENSOR engine: its slot is last in the
   walrus exit rendezvous, so the other engines' exit work overlaps the
   output-DMA completion latency.
"""
import numpy as np

import concourse.bass as bass
import concourse.mybir as mybir
from concourse.bass_utils import run_bass_kernel_spmd

F32 = mybir.dt.float32
F32R = mybir.dt.float32r
MULT = mybir.AluOpType.mult
SUB = mybir.AluOpType.subtract
ADD = mybir.AluOpType.add
SQUARE = mybir.ActivationFunctionType.Square
COPY = mybir.ActivationFunctionType.Copy

B, C, H, W = 2, 64, 32, 32
N = H * W          # 1024
NCHUNK = N // 4    # 256 output columns per core


TRIM_END_BARRIER = True  # drop the Block-exit barrier semaphores (keep drains)
HOIST_FRONT = True       # front of main (vs baseline's after-register-moves anchor)


def _hoist_to_front(nc: bass.Bass, insts) -> None:
    """Move the given instructions to the very front of the 'main' preamble
    block (right after the leading dummy call), preserving their relative
    order.  They execute as each engine's first instructions, so input-DMA
    completions land before the Block bodies open.  Hoisted instructions
    must have no semaphore waits."""
    main = nc.main_func.blocks[0]
    to_move = {id(i) for i in insts}
    for b in nc.main_func.blocks[1:]:
        b.instructions[:] = [i for i in b.instructions if id(i) not in to_move]
    main.instructions[:] = [i for i in main.instructions if id(i) not in to_move]
    pos = 1 if main.instructions and isinstance(main.instructions[0], mybir.InstCall) else 0
    for j, inst in enumerate(insts):
        main.instructions.insert(pos + j, inst)


def _hoist_after_moves(nc: bass.Bass, insts) -> None:
    """Baseline-style hoist: insert after each engine's last register-init
    move in 'main', preserving per-engine relative order."""
    main = nc.main_func.blocks[0]
    to_move = {id(i) for i in insts}
    for b in nc.main_func.blocks[1:]:
        b.instructions[:] = [i for i in b.instructions if id(i) not in to_move]
    main.instructions[:] = [i for i in main.instructions if id(i) not in to_move]
    anchor = {}
    for k, mi in enumerate(main.instructions):
        if isinstance(mi, mybir.InstRegisterMove):
            anchor[mi.engine] = k
    for inst in insts:
        idx = anchor.get(inst.engine)
        assert idx is not None, f"no register-move anchor for {inst.engine}"
        main.instructions.insert(idx + 1, inst)
        for eng in anchor:
            if anchor[eng] >= idx + 1:
                anchor[eng] += 1
        anchor[inst.engine] = idx + 1


def _build_nc() -> bass.Bass:
    nc = bass.Bass()
    x_ext = nc.declare_dram_parameter("xr", [128, 512], F32R, isOutput=False)
    w_ext = nc.declare_dram_parameter("w", [128, 192], F32R, isOutput=False)
    o_ext = nc.declare_dram_parameter("out", [64, 256], F32, isOutput=True)

    from contextlib import ExitStack

    with ExitStack() as ctx:
        e = ctx.enter_context
        # Wsb cols: 0:128 wkv (u/d weights, duplicated over partition halves)
        #           128:192 wq.T (parts 0:64) / wp.T (parts 64:128)
        Wsb = e(nc.sbuf_tensor("Wsb", [128, 192], F32R))
        Xsb = e(nc.sbuf_tensor("Xsb", [128, 512], F32R))
        cp2 = e(nc.sbuf_tensor("cp2", [128, 256], F32))   # SBUF copy of uv2
        sqs = e(nc.sbuf_tensor("sqs", [128, 256], F32))   # DVE square out (unread)
        redc = e(nc.sbuf_tensor("redc", [128, 4], F32))   # per-chunk row sums
        redall3 = e(nc.sbuf_tensor("redall3", [128, 1], F32))
        sdiff3 = e(nc.sbuf_tensor("sdiff3", [64, 1], F32))
        s4 = e(nc.sbuf_tensor("s4", [64, 1], F32))
        Qsb = e(nc.sbuf_tensor("Qsb", [64, 256], F32R))
        wpTs = e(nc.sbuf_tensor("wpTs", [64, 64], F32R))
        Fsb = e(nc.sbuf_tensor("Fsb", [64, 256], F32))
        uv1 = e(nc.psum_tensor("uv1", [128, 256], F32))
        uv2 = e(nc.psum_tensor("uv2", [128, 256], F32))
        uv3 = e(nc.psum_tensor("uv3", [128, 256], F32))
        uv4 = e(nc.psum_tensor("uv4", [128, 256], F32))
        sqp = e(nc.psum_tensor("sqp", [128, 256], F32))   # ACT square outs (unread)
        Qp = e(nc.psum_tensor("Qp", [64, 256], F32))
        Of = e(nc.psum_tensor("Of", [64, 256], F32))
        x1_sem = e(nc.semaphore("x1_sem"))
        x2_sem = e(nc.semaphore("x2_sem"))
        w_sem = e(nc.semaphore("w_sem"))
        pe_sem = e(nc.semaphore("pe_sem"))
        act_sem = e(nc.semaphore("act_sem"))
        dv_sem = e(nc.semaphore("dv_sem"))
        out_sem = e(nc.semaphore("out_sem"))
        block = e(nc.Block())

        def r(ap):
            return ap.bitcast(F32R)

        hoist = []

        @block.sync
        def _(sync):
            # second x half on the SP HWDGE ring, hoisted to main-front
            hoist.append(sync.dma_start(Xsb[:, 256:512], x_ext[:, 256:512]).then_inc(x2_sem, 16))
            # output chunk
            sync.wait_ge(dv_sem, 3)
            sync.dma_start(o_ext[:], Fsb[:]).then_inc(out_sem, 16)
            # Sync holds the out-DMA completion wait: its walrus exit slot is
            # ==4, so slots 1-3 complete during the DMA flight and only four
            # short slots + the semaphore resets trail the completion.
            sync.wait_ge(out_sem, 16)

        @block.tensor
        def _(pe):
            pe.wait_ge(w_sem, 16)
            pe.wait_ge(x1_sem, 16)
            # u,d: rows 0-63 = u = (wk+wv)x/2, rows 64-127 = d = (wk-wv)x/2
            pe.matmul(uv1[:], r(Wsb[0:64, 0:128]), r(Xsb[0:64, 0:256]), start=True, stop=True).then_inc(pe_sem, 1)
            pe.matmul(uv2[:], r(Wsb[64:128, 0:128]), r(Xsb[64:128, 0:256]), start=True, stop=True).then_inc(pe_sem, 1)
            pe.wait_ge(x2_sem, 16)
            pe.matmul(uv3[:], r(Wsb[0:64, 0:128]), r(Xsb[0:64, 256:512]), start=True, stop=True).then_inc(pe_sem, 1)
            pe.matmul(uv4[:], r(Wsb[64:128, 0:128]), r(Xsb[64:128, 256:512]), start=True, stop=True).then_inc(pe_sem, 1)
            # q for own chunk (cols 0:256 = own chunk, rotation puts it first);
            # q is only needed by the final matmul, so it runs after the uv
            # matmuls that feed the ACT square chain
            pe.matmul(Qp[:], r(Wsb[0:64, 128:192]), r(Xsb[0:64, 0:256]), start=True, stop=True).then_inc(pe_sem, 1)
            # out_attn = (wp diag(s)) @ q
            pe.wait_ge(dv_sem, 2)
            pe.matmul(Of[:], r(wpTs[:]), r(Qsb[:]), start=True, stop=True).then_inc(pe_sem, 1)

        @block.scalar
        def _(act):
            # w then first x half on the ACT ring (the earliest-released
            # engine stream); act table load behind them (all hoisted)
            hoist.append(act.dma_start(Wsb[:], w_ext[:]).then_inc(w_sem, 16))
            hoist.append(act.dma_start(Xsb[:, 0:256], x_ext[:, 0:256]).then_inc(x1_sem, 16))
            hoist.append(act.add_instruction(mybir.InstLoadActFuncSet(
                name=nc.get_next_instruction_name(), act_func_set_id=0, ins=[], outs=[],
            )))
            # fused square + row-sum for chunks 1, 3, 4
            act.wait_ge(pe_sem, 1)
            act.activation(sqp[:], uv1[:], SQUARE, accum_out=redc[:, 0:1]).then_inc(act_sem, 1)
            act.wait_ge(pe_sem, 3)
            act.activation(sqp[:], uv3[:], SQUARE, accum_out=redc[:, 2:3]).then_inc(act_sem, 1)
            act.wait_ge(pe_sem, 4)
            act.activation(sqp[:], uv4[:], SQUARE, accum_out=redc[:, 3:4]).then_inc(act_sem, 1)

        @block.vector
        def _(dv):
            dv.wait_ge(w_sem, 16)  # wpTs reads Wsb
            # chunk 2 square on DVE: PSUM->SBUF copy, then square+row-sum in
            # one scalar_tensor_tensor (out = (cp*1)*cp, accum = row sum)
            dv.wait_ge(pe_sem, 2)
            dv.tensor_copy(cp2[:], uv2[:])
            dv.scalar_tensor_tensor(sqs[:], cp2[:], 1.0, cp2[:], MULT, MULT, accum_out=redc[:, 1:2])
            # q copy PSUM->SBUF (with f32r rounding)
            dv.wait_ge(pe_sem, 5)
            dv.tensor_copy(Qsb[:], Qp[:]).then_inc(dv_sem, 1)
            dv.drain()  # own redc col 1 landed
            # pre-combine partials 1,2,3 while ACT squares chunk 4
            dv.wait_ge(act_sem, 2)
            dv.reduce_sum(redall3[:], redc[:, 0:3], axis=mybir.AxisListType.X)
            dv.drain()  # redall3 landed
            dv.tensor_scalar(sdiff3[:], redall3[0:64, :], redall3[64:128, :], None, op0=SUB)
            dv.drain()  # sdiff3 landed
            # s = (partial4_u - partial4_d) + sdiff3
            dv.wait_ge(act_sem, 3)
            dv.tensor_scalar(s4[:], redc[0:64, 3:4], redc[64:128, 3:4], sdiff3[:], op0=SUB, op1=ADD)
            dv.drain()  # s4 landed
            # wpTs = wp.T * s
            dv.tensor_scalar(wpTs[:], Wsb[64:128, 128:192], s4[:], None, op0=MULT).then_inc(dv_sem, 1)
            # out = out_attn + x for own chunk, PSUM -> SBUF
            dv.wait_ge(pe_sem, 6)
            dv.tensor_tensor(Fsb[:], Of[:], Xsb[0:64, 0:256], ADD).then_inc(dv_sem, 1)

    if HOIST_FRONT:
        _hoist_to_front(nc, [h.ins for h in hoist])
    else:
        _hoist_after_moves(nc, [h.ins for h in hoist])
    if TRIM_END_BARRIER:
        # the walrus postamble has its own all-engine rendezvous; the Block
        # exit barrier only delays it.  Keep the drains (write fences).
        end = nc.main_func.blocks[-1]
        end.instructions[:] = [
            i for i in end.instructions if not isinstance(i, mybir.InstEventSemaphore)
        ]
    return nc


def _shard_inputs(x, wq, wk, wv, wp):
    """Full inputs -> list of 8 per-core {'xr','w'} dicts."""
    x = np.asarray(x, dtype=np.float32)
    wq, wk, wv, wp = (np.asarray(a, dtype=np.float32) for a in (wq, wk, wv, wp))
    xf = np.ascontiguousarray(x.reshape(B, C, N))
    kv = np.concatenate([(wk + wv).T, (wk - wv).T], axis=1) * 0.5    # [64,128]
    wkv = np.concatenate([kv, kv], axis=0)                           # [128,128]
    wqp = np.concatenate([wq.T, wp.T], axis=0)                       # [128,64]
    wfull = np.ascontiguousarray(np.concatenate([wkv, wqp], axis=1))  # [128,192]
    in_maps = []
    for core in range(8):
        bb, j = core // 4, core % 4
        chunks = [xf[bb, :, ((j + t) % 4) * NCHUNK:(((j + t) % 4) + 1) * NCHUNK] for t in range(4)]
        upper = np.concatenate(chunks[0:2], axis=1)  # [64,512]
        lower = np.concatenate(chunks[2:4], axis=1)  # [64,512]
        xr = np.ascontiguousarray(np.concatenate([upper, lower], axis=0))  # [128,512]
        in_maps.append({"xr": xr, "w": wfull})
    return in_maps


def _gather_outputs(results):
    """8 per-core {'out': [64,256]} -> full [b,C,h,w]."""
    out = np.empty((B, C, N), dtype=np.float32)
    for core in range(8):
        bb, j = core // 4, core % 4
        out[bb, :, j * 256:(j + 1) * 256] = np.asarray(results[core]["out"])
    return out.reshape(B, C, H, W)


_NC_CACHE = None


def kernel(x, wq, wk, wv, wp) -> np.ndarray:
    global _NC_CACHE
    if _NC_CACHE is None:
        _NC_CACHE = _build_nc()
    in_maps = _shard_inputs(x, wq, wk, wv, wp)
    last_err = None
    for _ in range(3):
        try:
            res = run_bass_kernel_spmd(_NC_CACHE, in_maps, core_ids=list(range(8)))
            return _gather_outputs(res.results)
        except Exception as exc:  # transient device-unrecoverable resets on retry
            last_err = exc
    raise last_err
